# revision 1
# baseline (speedup 1.0000x reference)
import sys
sys.path.insert(0, '/opt/trn_rl_repo')
import numpy as np
import concourse.bacc as bacc_mod
import concourse.mybir as mybir
from concourse.tile import TileContext
from concourse.bass_utils import run_bass_kernel_spmd

F32 = mybir.dt.float32
F32R = mybir.dt.float32r
AF = mybir.ActivationFunctionType

B, IN, G, F, L0, L2 = 8192, 1024, 100, 128, 100, 50
GP, GPC = 104, 13          # padded groups, groups per core
NT, NW = 16, 512           # batch tiles
EPS = 1e-5
NCORES = 8

LAST_RESULTS = None
_prog_cache = None


def _r32r(a):
    """Round fp32 array to fp32r (11-bit mantissa, low 12 bits zero), RNE."""
    b = np.ascontiguousarray(a, dtype=np.float32).view(np.uint32).astype(np.uint64)
    b = (b + 0x800 + ((b >> 12) & 1)) & 0xFFFFF000
    return b.astype(np.uint32).view(np.float32)


def _gen_program():
    nc = bacc_mod.Bacc()
    xg_d = nc.declare_dram_parameter("xg", [GPC, F, B], F32R, isOutput=False)
    w0_d = nc.declare_dram_parameter("w0", [GPC, F, L0], F32R, isOutput=False)
    w1_d = nc.declare_dram_parameter("w1", [GPC, L0, L0], F32R, isOutput=False)
    w2_d = nc.declare_dram_parameter("w2", [GPC, L0 + 1, L2 + 1], F32R, isOutput=False)
    gb_d = nc.declare_dram_parameter("gb", [L0, 4 * GPC], F32, isOutput=False)
    onesb_d = nc.declare_dram_parameter("onesb", [1, B], F32R, isOutput=False)
    ones13_d = nc.declare_dram_parameter("ones13", [GPC, 1], F32R, isOutput=False)
    predT_d = nc.declare_dram_parameter("predT", [GPC, L2 + 1, B], F32, isOutput=True)
    cpart_d = nc.declare_dram_parameter("cpart", [NT, NW], F32, isOutput=True)

    with TileContext(nc) as tc:
        with tc.tile_pool(name="wp", bufs=1) as wp, \
             tc.tile_pool(name="xp", bufs=2) as xp, \
             tc.tile_pool(name="hp", bufs=1) as hp, \
             tc.tile_pool(name="sp", bufs=2) as sp, \
             tc.tile_pool(name="op", bufs=3) as op, \
             tc.tile_pool(name="ps0p", bufs=2, space="PSUM") as ps0p, \
             tc.tile_pool(name="ps1p", bufs=2, space="PSUM") as ps1p, \
             tc.tile_pool(name="ps2p", bufs=2, space="PSUM") as ps2p, \
             tc.tile_pool(name="cpp", bufs=2, space="PSUM") as cpp:
            w0_sb = wp.tile([F, GPC, L0], F32R)
            w1_sb = wp.tile([L0, GPC, L0], F32R)
            w2_sb = wp.tile([L0 + 1, GPC, L2 + 1], F32R)
            gb_sb = wp.tile([L0, 4 * GPC], F32)
            eps_sb = wp.tile([L0, 1], F32)
            nc.vector.memset(eps_sb[:, :], EPS)
            for g in range(GP // NCORES):
                nc.sync.dma_start(out=w0_sb[:, g, :], in_=w0_d[g, :, :])
                nc.sync.dma_start(out=w1_sb[:, g, :], in_=w1_d[g, :, :])
                nc.sync.dma_start(out=w2_sb[:, g, :], in_=w2_d[g, :, :])
            nc.sync.dma_start(out=gb_sb[:, :], in_=gb_d[:, :])
            h0r = hp.tile([L0, B], F32R)
            h1aug = hp.tile([L0 + 1, B], F32R)
            nc.sync.dma_start(out=h1aug[L0:L0 + 1, :], in_=onesb_d[:, :])

            for g in range(GPC):
                xg_sb = xp.tile([F, B], F32R, name=f"xg{g}", tag="xg")
                nc.sync.dma_start(out=xg_sb[:, :], in_=xg_d[g, :, :])
                # ---- layer0 pass A: stats
                st0 = sp.tile([L0, NT, 6], F32, name=f"st0_{g}", tag="st")
                for n in range(NT):
                    ps0 = ps0p.tile([L0, NW], F32, name=f"ps0_{g}_{n}", tag="ps0")
                    nc.tensor.matmul(ps0[:, :], w0_sb[:, g, :],
                                     xg_sb[:, n * NW:(n + 1) * NW],
                                     start=True, stop=True)
                    nc.vector.bn_stats(st0[:, n, :], ps0[:, :])
                mv0 = sp.tile([L0, 2], F32, name=f"mv0_{g}", tag="mv")
                nc.vector.bn_aggr(mv0[:, :], st0[:, :, :])
                r0 = sp.tile([L0, 1], F32, name=f"r0_{g}", tag="r")
                nc.scalar.activation(r0[:, :], mv0[:, 1:2], AF.Sqrt,
                                     bias=eps_sb[:, 0:1], scale=1.0)
                inv0 = sp.tile([L0, 1], F32, name=f"inv0_{g}", tag="inv")
                nc.vector.reciprocal(inv0[:, :], r0[:, :])
                s0 = sp.tile([L0, 1], F32, name=f"s0_{g}", tag="s")
                nc.vector.tensor_scalar_mul(s0[:, :], gb_sb[:, g:g + 1], inv0[:, 0:1])
                tm0 = sp.tile([L0, 1], F32, name=f"tm0_{g}", tag="tm")
                nc.vector.tensor_scalar_mul(tm0[:, :], mv0[:, 0:1], s0[:, 0:1])
                t0 = sp.tile([L0, 1], F32, name=f"t0_{g}", tag="t")
                nc.vector.tensor_scalar_sub(t0[:, :], gb_sb[:, GPC + g:GPC + g + 1],
                                            tm0[:, 0:1])
                # ---- layer0 pass B: recompute + BN + relu
                for n in range(NT):
                    ps0b = ps0p.tile([L0, NW], F32, name=f"ps0b_{g}_{n}", tag="ps0")
                    nc.tensor.matmul(ps0b[:, :], w0_sb[:, g, :],
                                     xg_sb[:, n * NW:(n + 1) * NW],
                                     start=True, stop=True)
                    nc.scalar.activation(h0r[:, n * NW:(n + 1) * NW], ps0b[:, :],
                                         AF.Relu, bias=t0[:, 0:1], scale=s0[:, 0:1])
                # ---- layer1 pass A: stats
                st1 = sp.tile([L0, NT, 6], F32, name=f"st1_{g}", tag="st")
                for n in range(NT):
                    ps1 = ps1p.tile([L0, NW], F32, name=f"ps1_{g}_{n}", tag="ps1")
                    nc.tensor.matmul(ps1[:, :], w1_sb[:, g, :],
                                     h0r[:, n * NW:(n + 1) * NW],
                                     start=True, stop=True)
                    nc.vector.bn_stats(st1[:, n, :], ps1[:, :])
                mv1 = sp.tile([L0, 2], F32, name=f"mv1_{g}", tag="mv")
                nc.vector.bn_aggr(mv1[:, :], st1[:, :, :])
                r1 = sp.tile([L0, 1], F32, name=f"r1_{g}", tag="r")
                nc.scalar.activation(r1[:, :], mv1[:, 1:2], AF.Sqrt,
                                     bias=eps_sb[:, 0:1], scale=1.0)
                inv1 = sp.tile([L0, 1], F32, name=f"inv1_{g}", tag="inv")
                nc.vector.reciprocal(inv1[:, :], r1[:, :])
                s1 = sp.tile([L0, 1], F32, name=f"s1_{g}", tag="s")
                nc.vector.tensor_scalar_mul(s1[:, :], gb_sb[:, 2 * GPC + g:2 * GPC + g + 1],
                                            inv1[:, 0:1])
                tm1 = sp.tile([L0, 1], F32, name=f"tm1_{g}", tag="tm")
                nc.vector.tensor_scalar_mul(tm1[:, :], mv1[:, 0:1], s1[:, 0:1])
                t1 = sp.tile([L0, 1], F32, name=f"t1_{g}", tag="t")
                nc.vector.tensor_scalar_sub(t1[:, :], gb_sb[:, 3 * GPC + g:3 * GPC + g + 1],
                                            tm1[:, 0:1])
                # ---- layer1 pass B + layer2 + store
                for n in range(NT):
                    ps1b = ps1p.tile([L0, NW], F32, name=f"ps1b_{g}_{n}", tag="ps1")
                    nc.tensor.matmul(ps1b[:, :], w1_sb[:, g, :],
                                     h0r[:, n * NW:(n + 1) * NW],
                                     start=True, stop=True)
                    nc.scalar.activation(h1aug[0:L0, n * NW:(n + 1) * NW], ps1b[:, :],
                                         AF.Relu, bias=t1[:, 0:1], scale=s1[:, 0:1])
                for n in range(NT):
                    ps2 = ps2p.tile([L2 + 1, NW], F32, name=f"ps2_{g}_{n}", tag="ps2")
                    nc.tensor.matmul(ps2[:, :], w2_sb[:, g, :],
                                     h1aug[:, n * NW:(n + 1) * NW],
                                     start=True, stop=True)
                    pout = op.tile([L2 + 1, NW], F32, name=f"po_{g}_{n}", tag="po")
                    nc.scalar.copy(pout[:, :], ps2[:, :])
                    nc.sync.dma_start(out=predT_d[g, :, n * NW:(n + 1) * NW],
                                      in_=pout[:, :])
            # ---- final: per-core sum of the 13 c-rows (row L2 of each group)
            tc.strict_bb_all_engine_barrier()
            cstage = wp.tile([GPC, B], F32R)
            nc.sync.dma_start(out=cstage[:, :],
                              in_=predT_d[:, L2:L2 + 1, :].bitcast(F32R))
            ones13 = wp.tile([GPC, 1], F32R)
            nc.sync.dma_start(out=ones13[:, :], in_=ones13_d[:, :])
            for n in range(NT):
                cps = cpp.tile([1, NW], F32, name=f"cps{n}", tag="cps")
                nc.tensor.matmul(cps[:, :], ones13[:, :],
                                 cstage[:, n * NW:(n + 1) * NW],
                                 start=True, stop=True)
                csb = op.tile([1, NW], F32, name=f"csb{n}", tag="csb", bufs=2)
                nc.scalar.copy(csb[:, :], cps[:, :])
                nc.sync.dma_start(out=cpart_d[n:n + 1, :], in_=csb[:, :])
    nc.compile()
    return nc


def kernel(x, used_features, w0, b0, w1, b1, w2, b2,
           gamma0, beta0, gamma1, beta1, out_weight, out_bias):
    global LAST_RESULTS, _prog_cache
    x = np.asarray(x, dtype=np.float32)
    uf = np.asarray(used_features, dtype=np.int64).reshape(G, F)
    ufp = np.zeros((GP, F), dtype=np.int64)
    ufp[:G] = uf
    # host layout prep (transpose/gather/pad/concat only; biases b0/b1 cancel in BN)
    xc = _r32r(np.ascontiguousarray(x.T))                     # [IN, B]
    w0p = np.zeros((GP, F, L0), np.float32); w0p[:G] = w0
    w1p = np.zeros((GP, L0, L0), np.float32); w1p[:G] = w1
    ow = np.asarray(out_weight, np.float32).reshape(G, L2)
    w2aug = np.zeros((GP, L0 + 1, L2 + 1), np.float32)
    w2aug[:G, :L0, :L2] = w2
    w2aug[:G, :L0, L2] = np.einsum('gij,gj->gi', np.asarray(w2, np.float32), ow)
    w2aug[:G, L0, :L2] = b2
    w2aug[:G, L0, L2] = np.einsum('gj,gj->g', np.asarray(b2, np.float32), ow)
    g0 = np.ones((GP, L0), np.float32); g0[:G] = np.asarray(gamma0, np.float32).reshape(G, L0)
    be0 = np.zeros((GP, L0), np.float32); be0[:G] = np.asarray(beta0, np.float32).reshape(G, L0)
    g1 = np.ones((GP, L0), np.float32); g1[:G] = np.asarray(gamma1, np.float32).reshape(G, L0)
    be1 = np.zeros((GP, L0), np.float32); be1[:G] = np.asarray(beta1, np.float32).reshape(G, L0)
    w0p = _r32r(w0p); w1p = _r32r(w1p); w2aug = _r32r(w2aug)
    onesb = np.ones((1, B), np.float32)
    ones13 = np.ones((GPC, 1), np.float32)

    in_maps = []
    for c in range(NCORES):
        s = slice(c * GPC, (c + 1) * GPC)
        xg = np.ascontiguousarray(xc[ufp[s]])                 # [GPC, F, B]
        gbcore = np.concatenate([g0[s].T, be0[s].T, g1[s].T, be1[s].T], axis=1)
        in_maps.append({
            "xg": xg,
            "w0": np.ascontiguousarray(w0p[s]),
            "w1": np.ascontiguousarray(w1p[s]),
            "w2": np.ascontiguousarray(w2aug[s]),
            "gb": np.ascontiguousarray(gbcore),
            "onesb": onesb,
            "ones13": ones13,
        })

    if _prog_cache is None:
        _prog_cache = _gen_program()
    res = run_bass_kernel_spmd(_prog_cache, in_maps, list(range(NCORES)))
    LAST_RESULTS = res

    predT = np.concatenate([res.results[c]["predT"] for c in range(NCORES)], axis=0)
    pred = np.ascontiguousarray(predT[:G, :L2, :].reshape(G * L2, B).T)
    csum = sum(res.results[c]["cpart"].reshape(B) for c in range(NCORES))
    logits = csum + np.float32(np.asarray(out_bias).reshape(-1)[0])
    out = (1.0 / (1.0 + np.exp(-logits.astype(np.float64)))).astype(np.float32)
    return out.reshape(B, 1), pred
